# revision 22
# baseline (speedup 1.0000x reference)
"""Laplace attention kernel for Trainium2, 8 NeuronCores.

Math (per batch b):
  k = MLP_k(x1[b])  [NK, D];  q = MLP_q(x2[b])  [NQ, D]
  dist[i,j] = sum_d |k[j,d] - q[i,d]|
  out = softmax_j(-dist) @ r[b]

Distribution: core c = (b, h) = (c//2, c%2): batch b, query-half h (256 queries).

Per-core algorithm:
  - MLPs run transposed on the PE: kT2 [128=(i2,d), NK] holds kT stacked twice,
    q2T [128=(i2,d), 128] holds qT for query pair p = (p, 128+p).
  - For each query pair p, a [128, NK] tile M_p is produced:
      min-form pairs (DVE):  M_p = min(kT2, q_p)        (tensor_scalar)
      abs-form pairs (ACT):  M_p = |kT2 - q_p|          (activation Abs)
    The PE reduces over the 128 partitions with a constant ones-block lhsT
    (coefficient -2 for min-form, +1 for abs-form), writing 2 PSUM rows per
    pair.  The reduction is COLUMN-TILED 4 ways: pair p -> col group
    g = p%4, psum rows 32g + 2s + {0,1} (s = slot), so 4 matmul streams run
    concurrently in the PE array (tile_position=(0, 32g)) - ~4x the serial
    column rate.  A K=1 correction matmul per (group, window) adds
    A_j = sum_d k[j,d] to min-form rows.  The remaining B_i offset is
    row-constant and cancels in softmax.
  - softmax: ACT Exp (scale=-1) per 512-window with accum_out row-sums;
    weights bf16.  Division by the sum happens on the host.
  - value matmul: PE-transpose the bf16 weights to [j, q] layout (8x
    [128,128]), evacuate to SBUF, then 8 accumulating matmuls with r as
    stationary operand -> [D, 128] PSUM; written out, transposed on host.
"""

import os
import numpy as np
import ml_dtypes

import concourse.bass as bass
import concourse.mybir as mybir
from concourse.tile import TileContext
from concourse import bass_utils

B, NQ, NK, D = 4, 512, 1024, 64
NCORES = 8
QSH = NQ // 2           # queries per core
NPAIR = QSH // 2        # 128 query pairs per core
NWIN = NK // 512        # 512-column matmul windows
NG = 4                  # PE column groups (tile_position)
NS = 16                 # slots per group per round

F32 = mybir.dt.float32
F16 = mybir.dt.float16
BF16 = mybir.dt.bfloat16

LAST_RESULT = None      # BassKernelResults of the most recent run (for test.py)


def _is_act_pair(p):
    # pair p = rr*64 + s*4 + g ; put col-group 3 on ACT except the earliest
    # slot of round 0 (ACT is still evacuating the MLPs / kt2 not complete)
    rr, m = divmod(p, 64)
    s, g = divmod(m, 4)
    return g == 3 and not (rr == 0 and s < 1)


# ---------------------------------------------------------------------------
# walrus workaround: the CTRL-class instructions (Drain etc.) can carry only a
# few sem waits; hoist excess waits onto injected NoOps on the same engine.
def _split_excess_waits(nc, max_waits=1):
    for f in nc.m.functions:
        for bb in f.blocks:
            new_insts = []
            for inst in bb.instructions:
                si = inst.sync_info
                if si is not None and si.on_wait and len(si.on_wait) > max_waits:
                    waits = list(si.on_wait)
                    excess, keep = waits[:-max_waits], waits[-max_waits:]
                    for i in range(0, len(excess), max_waits):
                        nop = mybir.InstNoOp(
                            name=f"{inst.name}_waitsplit_{i // max_waits}",
                            ins=[], outs=[])
                        nop.engine = inst.engine
                        nop.sync_info = mybir.SyncInfo(
                            on_wait=excess[i:i + max_waits], on_update=[])
                        new_insts.append(nop)
                    si.on_wait = keep
                new_insts.append(inst)
            bb.instructions = new_insts


# shim antenv.axon_hooks (absent in this image) so BASS_TRACE=1 profiling works
def _install_ntff_shim():
    import sys, types
    if 'antenv.axon_hooks' in sys.modules:
        return
    try:
        mod = types.ModuleType('antenv.axon_hooks')
        state = {}
        mod.set_axon_ntff_profile_hook = lambda h: state.__setitem__('h', h)
        mod.get_axon_ntff_profile_hook = lambda: state.get('h')
        sys.modules['antenv.axon_hooks'] = mod
        import antenv
        antenv.axon_hooks = mod
        from trn_agent_boot.trn_boot import _ntff_profile_via_ctypes
        h = _ntff_profile_via_ctypes('/opt/axon/libaxon_pjrt.so')
        if h is not None:
            mod.set_axon_ntff_profile_hook(h)
    except Exception:
        pass


# ---------------------------------------------------------------------------
def _build_program():
    nc = bass.Bass("TRN2")

    x1t = nc.dram_tensor("x1t", [D, NK], F16, kind="ExternalInput")
    x2t = nc.dram_tensor("x2t", [D, QSH], F16, kind="ExternalInput")
    rv = nc.dram_tensor("rv", [NK, D], BF16, kind="ExternalInput")
    wk1 = nc.dram_tensor("wk1", [D, D], F16, kind="ExternalInput")
    bk1 = nc.dram_tensor("bk1", [D, 1], F32, kind="ExternalInput")
    wk2d = nc.dram_tensor("wk2d", [D, 128], F16, kind="ExternalInput")
    bk2d = nc.dram_tensor("bk2d", [128, 1], F32, kind="ExternalInput")
    wq1 = nc.dram_tensor("wq1", [D, D], F16, kind="ExternalInput")
    bq1 = nc.dram_tensor("bq1", [D, 1], F32, kind="ExternalInput")
    wq2 = nc.dram_tensor("wq2", [D, D], F16, kind="ExternalInput")
    bq2d = nc.dram_tensor("bq2d", [128, 1], F32, kind="ExternalInput")
    wones = nc.dram_tensor("wones", [128, 32 * 32], F16, kind="ExternalInput")
    cmask = nc.dram_tensor("cmask", [1, 8 * 32], F16, kind="ExternalInput")
    ones64 = nc.dram_tensor("ones64", [D, 1], F16, kind="ExternalInput")
    ident = nc.dram_tensor("ident", [128, 128], BF16, kind="ExternalInput")
    ident32 = nc.dram_tensor("ident32", [128, 128], F32, kind="ExternalInput")
    yout = nc.dram_tensor("yout", [2, D, 128], F32, kind="ExternalOutput")
    sout = nc.dram_tensor("sout", [2, 2, 128], F32, kind="ExternalOutput")

    ACT = mybir.ActivationFunctionType
    ALU = mybir.AluOpType

    with TileContext(nc) as tc:
        import contextlib
        with contextlib.ExitStack() as ctx:
            consts = ctx.enter_context(tc.tile_pool(name="consts", bufs=1))

            x1t_sb = consts.tile([D, NK], F16)
            x2t_sb = consts.tile([D, QSH], F16)
            r_sb = consts.tile([128, 8 * D], BF16)
            wk1_sb = consts.tile([D, D], F16)
            bk1_sb = consts.tile([D, 1], F32)
            wk2d_sb = consts.tile([D, 128], F16)
            bk2d_sb = consts.tile([128, 1], F32)
            wq1_sb = consts.tile([D, D], F16)
            bq1_sb = consts.tile([D, 1], F32)
            wq2_sb = consts.tile([D, D], F16)
            bq2d_sb = consts.tile([128, 1], F32)
            wones_sb = consts.tile([128, 32 * 32], F16)
            cmask_sb = consts.tile([1, 8 * 32], F16)
            ones64_sb = consts.tile([D, 1], F16)
            ident_sb = consts.tile([128, 128], BF16)
            ident32_sb = consts.tile([128, 128], F32)
            dummy_sb = consts.tile([1, 16], F32)

            # trigger the ACT exp-table load (~1.3us) at t=0 on the scalar
            # queue, ahead of the first real activation
            nc.vector.memset(dummy_sb[:], 0.0)
            nc.scalar.activation(dummy_sb[:], dummy_sb[:],
                                 mybir.ActivationFunctionType.Exp,
                                 bias=0.0, scale=1.0)

            # input DMAs: 3 hw queues (sync / scalar=ACT / gpsimd), ordered so
            # the q-path and k-window-0 operands land first on each queue
            nc.sync.dma_start(out=wq1_sb[:], in_=wq1[:, :])
            nc.sync.dma_start(out=x2t_sb[:], in_=x2t[:, :])
            nc.sync.dma_start(out=wq2_sb[:], in_=wq2[:, :])
            nc.sync.dma_start(out=wones_sb[:], in_=wones[:, :])
            nc.scalar.dma_start(out=bq1_sb[:], in_=bq1[:, :])
            nc.scalar.dma_start(out=bq2d_sb[:], in_=bq2d[:, :])
            nc.gpsimd.dma_start(out=x1t_sb[:, 0:512], in_=x1t[:, 0:512])
            nc.gpsimd.dma_start(out=wk1_sb[:], in_=wk1[:, :])
            nc.gpsimd.dma_start(out=bk1_sb[:], in_=bk1[:, :])
            nc.gpsimd.dma_start(out=wk2d_sb[:], in_=wk2d[:, :])
            nc.gpsimd.dma_start(out=bk2d_sb[:], in_=bk2d[:, :])
            nc.gpsimd.dma_start(out=x1t_sb[:, 512:1024], in_=x1t[:, 512:1024])
            nc.gpsimd.dma_start(out=cmask_sb[:], in_=cmask[:, :])
            nc.gpsimd.dma_start(out=ones64_sb[:], in_=ones64[:, :])
            nc.gpsimd.dma_start(out=ident_sb[:], in_=ident[:, :])
            nc.gpsimd.dma_start(out=ident32_sb[:], in_=ident32[:, :])
            for jt in range(8):
                nc.gpsimd.dma_start(out=r_sb[:, jt * D:(jt + 1) * D],
                                    in_=rv[jt * 128:(jt + 1) * 128, :])

            kt2_sb = consts.tile([128, NK], F16)
            q2t_sb = consts.tile([128, 128], F32)
            ht_sb = consts.tile([D, NK], F16)
            hqt_sb = consts.tile([D, QSH], F16)
            arow_sb = consts.tile([1, NK], F16)

            # ---- MLPs (transposed) ----
            # q-path and k-window-0 interleaved so PE and ACT ping-pong with
            # no serial DMA wait; all evacuations on ACT so the DVE can start
            # min production immediately after.
            with tc.tile_pool(name="mlppsum", bufs=2, space="PSUM") as mp:
                phq = mp.tile([D, QSH], F32, tag="ph")
                nc.tensor.matmul(phq[:], wq1_sb[:], x2t_sb[:], start=True, stop=True)
                ph0 = mp.tile([D, 512], F32, tag="ph")
                nc.tensor.matmul(ph0[:], wk1_sb[:], x1t_sb[:, 0:512],
                                 start=True, stop=True)
                nc.scalar.activation(hqt_sb[:], phq[:], ACT.Relu,
                                     bias=bq1_sb[:, 0:1], scale=1.0)
                nc.scalar.activation(ht_sb[:, 0:512], ph0[:],
                                     ACT.Relu, bias=bk1_sb[:, 0:1], scale=1.0)
                pq = mp.tile([128, 128], F32, tag="pk")
                nc.tensor.matmul(pq[0:64, :], wq2_sb[:], hqt_sb[:, 0:128],
                                 start=True, stop=False, skip_group_check=True)
                nc.tensor.matmul(pq[64:128, :], wq2_sb[:], hqt_sb[:, 128:256],
                                 start=True, stop=True, skip_group_check=True)
                pk0 = mp.tile([128, 512], F32, tag="pk")
                nc.tensor.matmul(pk0[:], wk2d_sb[:], ht_sb[:, 0:512],
                                 start=True, stop=True)
                nc.scalar.activation(q2t_sb[:], pq[:], ACT.Identity,
                                     bias=bq2d_sb[:, 0:1], scale=1.0)
                nc.scalar.activation(kt2_sb[:, 0:512], pk0[:],
                                     ACT.Identity, bias=bk2d_sb[:, 0:1], scale=1.0)
                ph1 = mp.tile([D, 512], F32, tag="ph")
                nc.tensor.matmul(ph1[:], wk1_sb[:], x1t_sb[:, 512:1024],
                                 start=True, stop=True)
                nc.scalar.activation(ht_sb[:, 512:1024], ph1[:],
                                     ACT.Relu, bias=bk1_sb[:, 0:1], scale=1.0)
                pk1 = mp.tile([128, 512], F32, tag="pk")
                nc.tensor.matmul(pk1[:], wk2d_sb[:], ht_sb[:, 512:1024],
                                 start=True, stop=True)
                nc.scalar.activation(kt2_sb[:, 512:1024], pk1[:],
                                     ACT.Identity, bias=bk2d_sb[:, 0:1], scale=1.0)
                # A_j = sum_d k[j, d] (same fp16 k the min path sees)
                pa = mp.tile([1, NK], F32, tag="pa")
                for w in range(NWIN):
                    nc.tensor.matmul(pa[:, w * 512:(w + 1) * 512], ones64_sb[:],
                                     kt2_sb[0:64, w * 512:(w + 1) * 512],
                                     start=True, stop=True, skip_group_check=True)
                nc.scalar.copy(arow_sb[:], pa[:])

            # ---- main loop ----
            mpool = ctx.enter_context(tc.tile_pool(name="mtiles", bufs=8))
            dpool = ctx.enter_context(
                tc.tile_pool(name="dist", bufs=2, space="PSUM"))
            opool = ctx.enter_context(
                tc.tile_pool(name="outp", bufs=2, space="PSUM"))
            vpool = ctx.enter_context(
                tc.tile_pool(name="vps", bufs=1, space="PSUM"))
            spool = ctx.enter_context(tc.tile_pool(name="smax", bufs=2))
            otpool = ctx.enter_context(tc.tile_pool(name="outs", bufs=2))

            def make_tail(rr, dists):
                state = {}

                def exp_half(w):
                    if w == 0:
                        state["expw"] = spool.tile([128, NK], BF16, name="expw", tag="expw")
                        state["ssum"] = spool.tile([128, 2], F32, name="ssum", tag="ssum")
                        state["expt"] = spool.tile([128, 8 * 128], BF16, name="expt", tag="expt")
                    nc.scalar.activation(
                        state["expw"][:, w * 512:(w + 1) * 512],
                        dists[w][:], ACT.Exp,
                        bias=0.0, scale=-1.0,
                        accum_out=state["ssum"][:, w:w + 1])

                def tp_pair(h):
                    # transpose 2 of the 8 [128,128] blocks on the PE;
                    # evacuate psum->sbuf on ACT (it has slack; DVE is the
                    # critical engine)
                    expw, expt = state["expw"], state["expt"]
                    for jt in (2 * h, 2 * h + 1):
                        tp = opool.tile([128, 128], BF16, tag="tp")
                        nc.tensor.transpose(tp[:], expw[:, jt * 128:(jt + 1) * 128],
                                            ident_sb[:])
                        nc.scalar.copy(expt[:, jt * 128:(jt + 1) * 128], tp[:])

                def value():
                    expt = state["expt"]
                    ssum = state["ssum"]
                    # partition-dim ssum -> free-dim via PE transpose, then a
                    # linear 2-partition DMA (a direct [128,1]-strided DMA
                    # costs 128 descriptors and lands ~5us late)
                    stp = vpool.tile([2, 128], F32, tag="stp")
                    nc.tensor.transpose(stp[:], ssum[:, 0:2], ident32_sb[:])
                    sst = otpool.tile([2, 128], F32, tag="sst")
                    nc.scalar.copy(sst[:], stp[:])
                    nc.gpsimd.dma_start(out=sout[rr, :, :], in_=sst[:])
                    out_ps = vpool.tile([D, 128], F32, tag="ops")
                    for jt in range(8):
                        nc.tensor.matmul(out_ps[:, :],
                                         r_sb[:, jt * D:(jt + 1) * D],
                                         expt[:, jt * 128:(jt + 1) * 128],
                                         start=(jt == 0), stop=(jt == 7),
                                         skip_group_check=True)
                    ot = otpool.tile([D, 128], F32, tag="ot")
                    nc.scalar.copy(ot[:], out_ps[:])
                    nc.sync.dma_start(out=yout[rr, :, :], in_=ot[:])
                return exp_half, tp_pair, value

            prev = None
            for rr in range(2):
                dists = [dpool.tile([128, 512], F32, name=f"dist{w}",
                                    tag=f"dist{w}") for w in range(NWIN)]
                for s in range(NS):
                    for g in range(NG):
                        p = rr * 64 + s * 4 + g
                        mt = mpool.tile([128, NK], F16, tag="mt")
                        if _is_act_pair(p):
                            nc.scalar.activation(mt[:], kt2_sb[:], ACT.Abs,
                                                 bias=q2t_sb[:, p:p + 1], scale=-1.0)
                            bi = 16 + s
                        elif rr == 0 and s < 2:
                            # per-window halves: lets window-0 matmuls start
                            # before the second kt2 window is computed
                            for w in range(NWIN):
                                nc.vector.tensor_scalar(
                                    mt[:, w * 512:(w + 1) * 512],
                                    kt2_sb[:, w * 512:(w + 1) * 512],
                                    q2t_sb[:, p:p + 1], None, ALU.min)
                            bi = s
                        else:
                            nc.vector.tensor_scalar(mt[:], kt2_sb[:],
                                                    q2t_sb[:, p:p + 1], None, ALU.min)
                            bi = s
                        for w in range(NWIN):
                            nc.tensor.matmul(
                                dists[w][32 * g:32 * g + 32, :],
                                wones_sb[:, bi * 32:(bi + 1) * 32],
                                mt[:, w * 512:(w + 1) * 512],
                                start=(s == 0), stop=False,
                                skip_group_check=True,
                                tile_position=(0, 32 * g))
                    if prev is not None:
                        if s in (0, 1, 2, 3):
                            prev[1](s)        # prev-round transposes
                        elif s == 5:
                            prev[2]()         # value matmuls + out DMA
                            prev = None
                # A_j correction (min-form rows only), closes each psum
                # window; exp of a window issues as soon as it closes
                cur = make_tail(rr, dists)
                for w in range(NWIN):
                    for g in range(NG):
                        cb = rr * 4 + g
                        nc.tensor.matmul(
                            dists[w][32 * g:32 * g + 32, :],
                            cmask_sb[:, cb * 32:(cb + 1) * 32],
                            arow_sb[:, w * 512:(w + 1) * 512],
                            start=False, stop=True,
                            skip_group_check=True,
                            tile_position=(0, 32 * g))
                    cur[0](w)
                prev = cur
            # tail of the final round (exp halves already issued above)
            for h in range(4):
                prev[1](h)
            prev[2]()

    _split_excess_waits(nc)
    return nc


_NC_CACHE = None


def _get_nc():
    global _NC_CACHE
    if _NC_CACHE is None:
        _NC_CACHE = _build_program()
    return _NC_CACHE


def kernel(x1, x2, r, Wk1, bk1, Wk2, bk2, Wq1, bq1, Wq2, bq2):
    global LAST_RESULT
    x1 = np.asarray(x1, np.float32)
    x2 = np.asarray(x2, np.float32)
    r = np.asarray(r, np.float32)
    Wk1 = np.asarray(Wk1, np.float32); bk1 = np.asarray(bk1, np.float32)
    Wk2 = np.asarray(Wk2, np.float32); bk2 = np.asarray(bk2, np.float32)
    Wq1 = np.asarray(Wq1, np.float32); bq1 = np.asarray(bq1, np.float32)
    Wq2 = np.asarray(Wq2, np.float32); bq2 = np.asarray(bq2, np.float32)

    # constant PE weights: ones-block lhsT [128, 32] per (form, slot); block
    # b = form*16 + s maps partition halves to psum rows (2s, 2s+1) within a
    # col group, coefficient -2 (min-form) / +1 (abs-form). cmask: per
    # (round, group) A_j-correction masks over the group's 32 psum rows.
    wones = np.zeros((128, 32 * 32), np.float32)
    for s in range(16):
        wones[0:64, s * 32 + 2 * s] = -2.0
        wones[64:128, s * 32 + 2 * s + 1] = -2.0
        wones[0:64, (16 + s) * 32 + 2 * s] = 1.0
        wones[64:128, (16 + s) * 32 + 2 * s + 1] = 1.0
    cmask = np.zeros((1, 8 * 32), np.float32)
    for rr in range(2):
        for s in range(16):
            for g in range(4):
                p = rr * 64 + s * 4 + g
                if not _is_act_pair(p):
                    cb = rr * 4 + g
                    cmask[0, cb * 32 + 2 * s] = 1.0
                    cmask[0, cb * 32 + 2 * s + 1] = 1.0
    shared = {
        "wk1": Wk1.astype(np.float16), "bk1": bk1.reshape(D, 1),
        "wk2d": np.concatenate([Wk2, Wk2], axis=1).astype(np.float16),
        "bk2d": np.concatenate([bk2, bk2]).reshape(128, 1),
        "wq1": Wq1.astype(np.float16), "bq1": bq1.reshape(D, 1),
        "wq2": Wq2.astype(np.float16),
        "bq2d": np.concatenate([bq2, bq2]).reshape(128, 1),
        "wones": wones.astype(np.float16), "cmask": cmask.astype(np.float16),
        "ones64": np.ones((D, 1), np.float16),
        "ident": np.eye(128, dtype=ml_dtypes.bfloat16),
        "ident32": np.eye(128, dtype=np.float32),
    }
    shared = {k: np.ascontiguousarray(v) for k, v in shared.items()}

    in_maps = []
    for c in range(NCORES):
        b, h = c // 2, c % 2
        m = dict(shared)
        m["x1t"] = np.ascontiguousarray(x1[b].T.astype(np.float16))
        m["x2t"] = np.ascontiguousarray(x2[b, h * QSH:(h + 1) * QSH].T.astype(np.float16))
        m["rv"] = np.ascontiguousarray(r[b].astype(ml_dtypes.bfloat16))
        in_maps.append(m)

    nc = _get_nc()
    trace = bool(os.environ.get("BASS_TRACE"))
    if trace:
        _install_ntff_shim()
    res = None
    for attempt in range(3):
        try:
            res = bass_utils.run_bass_kernel_spmd(
                nc, in_maps, core_ids=list(range(NCORES)), trace=trace)
            break
        except Exception:
            # transient NRT_EXEC_UNIT_UNRECOVERABLE failures have been
            # observed on this fabric; retry (compile results are cached)
            if attempt == 2:
                raise
            import time
            time.sleep(5)
    LAST_RESULT = res

    # reassemble: yout[rr, f, t] with psum row t = 32g + 2s + i2,
    # local query = i2*128 + rr*64 + s*4 + g
    t = np.arange(128)
    g = t // 32
    m = t % 32
    s = m // 2
    i2 = m % 2
    out = np.empty((B, NQ, D), np.float32)
    for c in range(NCORES):
        b, h = c // 2, c % 2
        yc = res.results[c]["yout"]          # [2, D, 128]
        sc = res.results[c]["sout"]          # [2, 2, 128]
        for rr in range(2):
            qloc = i2 * 128 + rr * 64 + s * 4 + g
            ssum = sc[rr, 0] + sc[rr, 1]
            out[b, h * QSH + qloc, :] = (yc[rr] / ssum[None, :]).T
    return out


# revision 24
# speedup vs baseline: 1.2508x; 1.2508x over previous
"""Laplace attention kernel for Trainium2, 8 NeuronCores.

Math (per batch b):
  k = MLP_k(x1[b])  [NK, D];  q = MLP_q(x2[b])  [NQ, D]
  dist[i,j] = sum_d |k[j,d] - q[i,d]|
  out = softmax_j(-dist) @ r[b]

Distribution: core c = (b, h) = (c//2, c%2): batch b, query-half h (256 queries).

Per-core algorithm:
  - MLPs run transposed on the PE: kT2 [128=(i2,d), NK] holds kT stacked twice,
    q2T [128=(i2,d), 128] holds qT for query pair p = (p, 128+p).
  - For each query pair p, a [128, NK] tile M_p is produced:
      min-form pairs (DVE):  M_p = min(kT2, q_p)        (tensor_scalar)
      abs-form pairs (ACT):  M_p = |kT2 - q_p|          (activation Abs)
    The PE reduces over the 128 partitions with a constant ones-block lhsT
    (coefficient -2 for min-form, +1 for abs-form), writing 2 PSUM rows per
    pair.  The reduction is COLUMN-TILED 4 ways: pair p -> col group
    g = p%4, psum rows 32g + 2s + {0,1} (s = slot), so 4 matmul streams run
    concurrently in the PE array (tile_position=(0, 32g)) - ~4x the serial
    column rate.  A K=1 correction matmul per (group, window) adds
    A_j = sum_d k[j,d] to min-form rows.  The remaining B_i offset is
    row-constant and cancels in softmax.
  - softmax: ACT Exp (scale=-1) per 512-window with accum_out row-sums;
    weights bf16.  Division by the sum happens on the host.
  - value matmul: PE-transpose the bf16 weights to [j, q] layout (8x
    [128,128]), evacuate to SBUF, then 8 accumulating matmuls with r as
    stationary operand -> [D, 128] PSUM; written out, transposed on host.
"""

import os
import numpy as np
import ml_dtypes

import concourse.bass as bass
import concourse.mybir as mybir
from concourse.tile import TileContext
from concourse import bass_utils

B, NQ, NK, D = 4, 512, 1024, 64
NCORES = 8
QSH = NQ // 2           # queries per core
NPAIR = QSH // 2        # 128 query pairs per core
NWIN = NK // 512        # 512-column matmul windows
NG = 4                  # PE column groups (tile_position)
NS = 16                 # slots per group per round

F32 = mybir.dt.float32
F16 = mybir.dt.float16
BF16 = mybir.dt.bfloat16

LAST_RESULT = None      # BassKernelResults of the most recent run (for test.py)


def _is_act_pair(p):
    # pair p = rr*64 + s*4 + g ; put col-group 3 on ACT except the earliest
    # slot of round 0 (ACT is still evacuating the MLPs / kt2 not complete)
    rr, m = divmod(p, 64)
    s, g = divmod(m, 4)
    return g == 3 and not (rr == 0 and s < 1)


# ---------------------------------------------------------------------------
# walrus workaround: the CTRL-class instructions (Drain etc.) can carry only a
# few sem waits; hoist excess waits onto injected NoOps on the same engine.
def _split_excess_waits(nc, max_waits=1):
    for f in nc.m.functions:
        for bb in f.blocks:
            new_insts = []
            for inst in bb.instructions:
                si = inst.sync_info
                if si is not None and si.on_wait and len(si.on_wait) > max_waits:
                    waits = list(si.on_wait)
                    excess, keep = waits[:-max_waits], waits[-max_waits:]
                    for i in range(0, len(excess), max_waits):
                        nop = mybir.InstNoOp(
                            name=f"{inst.name}_waitsplit_{i // max_waits}",
                            ins=[], outs=[])
                        nop.engine = inst.engine
                        nop.sync_info = mybir.SyncInfo(
                            on_wait=excess[i:i + max_waits], on_update=[])
                        new_insts.append(nop)
                    si.on_wait = keep
                new_insts.append(inst)
            bb.instructions = new_insts


# shim antenv.axon_hooks (absent in this image) so BASS_TRACE=1 profiling works
def _install_ntff_shim():
    import sys, types
    if 'antenv.axon_hooks' in sys.modules:
        return
    try:
        mod = types.ModuleType('antenv.axon_hooks')
        state = {}
        mod.set_axon_ntff_profile_hook = lambda h: state.__setitem__('h', h)
        mod.get_axon_ntff_profile_hook = lambda: state.get('h')
        sys.modules['antenv.axon_hooks'] = mod
        import antenv
        antenv.axon_hooks = mod
        from trn_agent_boot.trn_boot import _ntff_profile_via_ctypes
        h = _ntff_profile_via_ctypes('/opt/axon/libaxon_pjrt.so')
        if h is not None:
            mod.set_axon_ntff_profile_hook(h)
    except Exception:
        pass


# ---------------------------------------------------------------------------
def _build_program():
    nc = bass.Bass("TRN2")

    x1t = nc.dram_tensor("x1t", [D, NK], F16, kind="ExternalInput")
    x2t = nc.dram_tensor("x2t", [D, QSH], F16, kind="ExternalInput")
    rv = nc.dram_tensor("rv", [NK, D], BF16, kind="ExternalInput")
    wk1 = nc.dram_tensor("wk1", [D, D], F16, kind="ExternalInput")
    bk1 = nc.dram_tensor("bk1", [D, 1], F32, kind="ExternalInput")
    wk2d = nc.dram_tensor("wk2d", [D, 128], F16, kind="ExternalInput")
    bk2d = nc.dram_tensor("bk2d", [128, 1], F32, kind="ExternalInput")
    wq1 = nc.dram_tensor("wq1", [D, D], F16, kind="ExternalInput")
    bq1 = nc.dram_tensor("bq1", [D, 1], F32, kind="ExternalInput")
    wq2 = nc.dram_tensor("wq2", [D, D], F16, kind="ExternalInput")
    bq2d = nc.dram_tensor("bq2d", [128, 1], F32, kind="ExternalInput")
    wones = nc.dram_tensor("wones", [128, 32 * 32], F16, kind="ExternalInput")
    cmask = nc.dram_tensor("cmask", [1, 8 * 32], F16, kind="ExternalInput")
    ones64 = nc.dram_tensor("ones64", [D, 1], F16, kind="ExternalInput")
    ident = nc.dram_tensor("ident", [128, 128], BF16, kind="ExternalInput")
    ident32 = nc.dram_tensor("ident32", [128, 128], F32, kind="ExternalInput")
    yout = nc.dram_tensor("yout", [2, D, 128], F32, kind="ExternalOutput")
    sout = nc.dram_tensor("sout", [2, 2, 128], F32, kind="ExternalOutput")

    ACT = mybir.ActivationFunctionType
    ALU = mybir.AluOpType

    with TileContext(nc) as tc:
        import contextlib
        with contextlib.ExitStack() as ctx:
            consts = ctx.enter_context(tc.tile_pool(name="consts", bufs=1))

            x1t_sb = consts.tile([D, NK], F16)
            x2t_sb = consts.tile([D, QSH], F16)
            r_sb = consts.tile([128, 8 * D], BF16)
            wk1_sb = consts.tile([D, D], F16)
            bk1_sb = consts.tile([D, 1], F32)
            wk2d_sb = consts.tile([D, 128], F16)
            bk2d_sb = consts.tile([128, 1], F32)
            wq1_sb = consts.tile([D, D], F16)
            bq1_sb = consts.tile([D, 1], F32)
            wq2_sb = consts.tile([D, D], F16)
            bq2d_sb = consts.tile([128, 1], F32)
            wones_sb = consts.tile([128, 32 * 32], F16)
            cmask_sb = consts.tile([1, 8 * 32], F16)
            ones64_sb = consts.tile([D, 1], F16)
            ident_sb = consts.tile([128, 128], BF16)
            ident32_sb = consts.tile([128, 128], F32)
            dummy_sb = consts.tile([1, 16], F32)

            # trigger the ACT exp-table load (~1.3us) at t=0 on the scalar
            # queue, ahead of the first real activation
            nc.vector.memset(dummy_sb[:], 0.0)
            nc.scalar.activation(dummy_sb[:], dummy_sb[:],
                                 mybir.ActivationFunctionType.Exp,
                                 bias=0.0, scale=1.0)

            # input DMAs: 3 hw queues (sync / scalar=ACT / gpsimd), ordered so
            # the q-path and k-window-0 operands land first on each queue
            nc.sync.dma_start(out=wq1_sb[:], in_=wq1[:, :])
            nc.sync.dma_start(out=x2t_sb[:], in_=x2t[:, :])
            nc.sync.dma_start(out=wq2_sb[:], in_=wq2[:, :])
            nc.sync.dma_start(out=wones_sb[:], in_=wones[:, :])
            nc.scalar.dma_start(out=bq1_sb[:], in_=bq1[:, :])
            nc.scalar.dma_start(out=bq2d_sb[:], in_=bq2d[:, :])
            nc.gpsimd.dma_start(out=x1t_sb[:, 0:512], in_=x1t[:, 0:512])
            nc.gpsimd.dma_start(out=wk1_sb[:], in_=wk1[:, :])
            nc.gpsimd.dma_start(out=bk1_sb[:], in_=bk1[:, :])
            nc.gpsimd.dma_start(out=wk2d_sb[:], in_=wk2d[:, :])
            nc.gpsimd.dma_start(out=bk2d_sb[:], in_=bk2d[:, :])
            nc.gpsimd.dma_start(out=x1t_sb[:, 512:1024], in_=x1t[:, 512:1024])
            nc.gpsimd.dma_start(out=cmask_sb[:], in_=cmask[:, :])
            nc.gpsimd.dma_start(out=ones64_sb[:], in_=ones64[:, :])
            nc.gpsimd.dma_start(out=ident_sb[:], in_=ident[:, :])
            nc.gpsimd.dma_start(out=ident32_sb[:], in_=ident32[:, :])
            for jt in range(8):
                nc.gpsimd.dma_start(out=r_sb[:, jt * D:(jt + 1) * D],
                                    in_=rv[jt * 128:(jt + 1) * 128, :])

            kt2_sb = consts.tile([128, NK], F16)
            q2t_sb = consts.tile([128, 128], F32)
            ht_sb = consts.tile([D, NK], F16)
            hqt_sb = consts.tile([D, QSH], F16)
            arow_sb = consts.tile([1, NK], F16)

            # ---- MLPs (transposed) ----
            # q-path and k-window-0 interleaved so PE and ACT ping-pong with
            # no serial DMA wait; all evacuations on ACT so the DVE can start
            # min production immediately after.
            with tc.tile_pool(name="mlppsum", bufs=2, space="PSUM") as mp:
                phq = mp.tile([D, QSH], F32, tag="ph")
                nc.tensor.matmul(phq[:], wq1_sb[:], x2t_sb[:], start=True, stop=True)
                ph0 = mp.tile([D, 512], F32, tag="ph")
                nc.tensor.matmul(ph0[:], wk1_sb[:], x1t_sb[:, 0:512],
                                 start=True, stop=True)
                nc.scalar.activation(hqt_sb[:], phq[:], ACT.Relu,
                                     bias=bq1_sb[:, 0:1], scale=1.0)
                nc.scalar.activation(ht_sb[:, 0:512], ph0[:],
                                     ACT.Relu, bias=bk1_sb[:, 0:1], scale=1.0)
                pq = mp.tile([128, 128], F32, tag="pk")
                nc.tensor.matmul(pq[0:64, :], wq2_sb[:], hqt_sb[:, 0:128],
                                 start=True, stop=False, skip_group_check=True)
                nc.tensor.matmul(pq[64:128, :], wq2_sb[:], hqt_sb[:, 128:256],
                                 start=True, stop=True, skip_group_check=True)
                pk0 = mp.tile([128, 512], F32, tag="pk")
                nc.tensor.matmul(pk0[:], wk2d_sb[:], ht_sb[:, 0:512],
                                 start=True, stop=True)
                nc.scalar.activation(q2t_sb[:], pq[:], ACT.Identity,
                                     bias=bq2d_sb[:, 0:1], scale=1.0)
                nc.scalar.activation(kt2_sb[:, 0:512], pk0[:],
                                     ACT.Identity, bias=bk2d_sb[:, 0:1], scale=1.0)
                ph1 = mp.tile([D, 512], F32, tag="ph")
                nc.tensor.matmul(ph1[:], wk1_sb[:], x1t_sb[:, 512:1024],
                                 start=True, stop=True)
                nc.scalar.activation(ht_sb[:, 512:1024], ph1[:],
                                     ACT.Relu, bias=bk1_sb[:, 0:1], scale=1.0)
                pk1 = mp.tile([128, 512], F32, tag="pk")
                nc.tensor.matmul(pk1[:], wk2d_sb[:], ht_sb[:, 512:1024],
                                 start=True, stop=True)
                nc.scalar.activation(kt2_sb[:, 512:1024], pk1[:],
                                     ACT.Identity, bias=bk2d_sb[:, 0:1], scale=1.0)
                # A_j = sum_d k[j, d] (same fp16 k the min path sees)
                pa = mp.tile([1, NK], F32, tag="pa")
                for w in range(NWIN):
                    nc.tensor.matmul(pa[:, w * 512:(w + 1) * 512], ones64_sb[:],
                                     kt2_sb[0:64, w * 512:(w + 1) * 512],
                                     start=True, stop=True, skip_group_check=True)
                nc.scalar.copy(arow_sb[:], pa[:])

            # ---- main loop ----
            mpool = ctx.enter_context(tc.tile_pool(name="mtiles", bufs=8))
            dpool = ctx.enter_context(
                tc.tile_pool(name="dist", bufs=2, space="PSUM"))
            opool = ctx.enter_context(
                tc.tile_pool(name="outp", bufs=2, space="PSUM"))
            vpool = ctx.enter_context(
                tc.tile_pool(name="vps", bufs=1, space="PSUM"))
            spool = ctx.enter_context(tc.tile_pool(name="smax", bufs=2))
            otpool = ctx.enter_context(tc.tile_pool(name="outs", bufs=2))

            def make_tail(rr, dists):
                state = {}

                def exp_half(w):
                    if w == 0:
                        state["expw"] = spool.tile([128, NK], BF16, name="expw", tag="expw")
                        state["ssum"] = spool.tile([128, 2], F32, name="ssum", tag="ssum")
                        state["expt"] = spool.tile([128, 8 * 128], BF16, name="expt", tag="expt")
                    nc.scalar.activation(
                        state["expw"][:, w * 512:(w + 1) * 512],
                        dists[w][:], ACT.Exp,
                        bias=0.0, scale=-1.0,
                        accum_out=state["ssum"][:, w:w + 1])

                def tp_pair(h):
                    # transpose 2 of the 8 [128,128] blocks on the PE;
                    # evacuations split DVE/ACT so the tp psum ring (bufs=2)
                    # turns over fast enough not to stall the PE queue
                    expw, expt = state["expw"], state["expt"]
                    for jt in (2 * h, 2 * h + 1):
                        tp = opool.tile([128, 128], BF16, tag="tp")
                        nc.tensor.transpose(tp[:], expw[:, jt * 128:(jt + 1) * 128],
                                            ident_sb[:])
                        if jt % 2 == 0:
                            nc.vector.tensor_copy(
                                expt[:, jt * 128:(jt + 1) * 128], tp[:])
                        else:
                            nc.scalar.copy(
                                expt[:, jt * 128:(jt + 1) * 128], tp[:])

                def value():
                    expt = state["expt"]
                    ssum = state["ssum"]
                    # partition-dim ssum -> free-dim via PE transpose, then a
                    # linear 2-partition DMA (a direct [128,1]-strided DMA
                    # costs 128 descriptors and lands ~5us late)
                    stp = vpool.tile([2, 128], F32, tag="stp")
                    nc.tensor.transpose(stp[:], ssum[:, 0:2], ident32_sb[:])
                    sst = otpool.tile([2, 128], F32, tag="sst")
                    nc.scalar.copy(sst[:], stp[:])
                    nc.gpsimd.dma_start(out=sout[rr, :, :], in_=sst[:])
                    out_ps = vpool.tile([D, 128], F32, tag="ops")
                    for jt in range(8):
                        nc.tensor.matmul(out_ps[:, :],
                                         r_sb[:, jt * D:(jt + 1) * D],
                                         expt[:, jt * 128:(jt + 1) * 128],
                                         start=(jt == 0), stop=(jt == 7),
                                         skip_group_check=True)
                    ot = otpool.tile([D, 128], F32, tag="ot")
                    nc.scalar.copy(ot[:], out_ps[:])
                    nc.sync.dma_start(out=yout[rr, :, :], in_=ot[:])
                return exp_half, tp_pair, value

            prev = None
            for rr in range(2):
                dists = [dpool.tile([128, 512], F32, name=f"dist{w}",
                                    tag=f"dist{w}") for w in range(NWIN)]
                for s in range(NS):
                    for g in range(NG):
                        p = rr * 64 + s * 4 + g
                        mt = mpool.tile([128, NK], F16, tag="mt")
                        if _is_act_pair(p):
                            nc.scalar.activation(mt[:], kt2_sb[:], ACT.Abs,
                                                 bias=q2t_sb[:, p:p + 1], scale=-1.0)
                            bi = 16 + s
                        elif rr == 0 and s < 2:
                            # per-window halves: lets window-0 matmuls start
                            # before the second kt2 window is computed
                            for w in range(NWIN):
                                nc.vector.tensor_scalar(
                                    mt[:, w * 512:(w + 1) * 512],
                                    kt2_sb[:, w * 512:(w + 1) * 512],
                                    q2t_sb[:, p:p + 1], None, ALU.min)
                            bi = s
                        else:
                            nc.vector.tensor_scalar(mt[:], kt2_sb[:],
                                                    q2t_sb[:, p:p + 1], None, ALU.min)
                            bi = s
                        for w in range(NWIN):
                            nc.tensor.matmul(
                                dists[w][32 * g:32 * g + 32, :],
                                wones_sb[:, bi * 32:(bi + 1) * 32],
                                mt[:, w * 512:(w + 1) * 512],
                                start=(s == 0), stop=False,
                                skip_group_check=True,
                                tile_position=(0, 32 * g))
                    if prev is not None:
                        if s in (1, 2, 3, 4):
                            prev[1](s - 1)    # prev-round transposes
                        elif s == 6:
                            prev[2]()         # value matmuls + out DMA
                            prev = None
                # A_j correction (min-form rows only), closes each psum
                # window; exp of a window issues as soon as it closes
                cur = make_tail(rr, dists)
                for w in range(NWIN):
                    for g in range(NG):
                        cb = rr * 4 + g
                        nc.tensor.matmul(
                            dists[w][32 * g:32 * g + 32, :],
                            cmask_sb[:, cb * 32:(cb + 1) * 32],
                            arow_sb[:, w * 512:(w + 1) * 512],
                            start=False, stop=True,
                            skip_group_check=True,
                            tile_position=(0, 32 * g))
                    cur[0](w)
                prev = cur
            # tail of the final round (exp halves already issued above)
            for h in range(4):
                prev[1](h)
            prev[2]()

    _split_excess_waits(nc)
    return nc


_NC_CACHE = None


def _get_nc():
    global _NC_CACHE
    if _NC_CACHE is None:
        _NC_CACHE = _build_program()
    return _NC_CACHE


def kernel(x1, x2, r, Wk1, bk1, Wk2, bk2, Wq1, bq1, Wq2, bq2):
    global LAST_RESULT
    x1 = np.asarray(x1, np.float32)
    x2 = np.asarray(x2, np.float32)
    r = np.asarray(r, np.float32)
    Wk1 = np.asarray(Wk1, np.float32); bk1 = np.asarray(bk1, np.float32)
    Wk2 = np.asarray(Wk2, np.float32); bk2 = np.asarray(bk2, np.float32)
    Wq1 = np.asarray(Wq1, np.float32); bq1 = np.asarray(bq1, np.float32)
    Wq2 = np.asarray(Wq2, np.float32); bq2 = np.asarray(bq2, np.float32)

    # constant PE weights: ones-block lhsT [128, 32] per (form, slot); block
    # b = form*16 + s maps partition halves to psum rows (2s, 2s+1) within a
    # col group, coefficient -2 (min-form) / +1 (abs-form). cmask: per
    # (round, group) A_j-correction masks over the group's 32 psum rows.
    wones = np.zeros((128, 32 * 32), np.float32)
    for s in range(16):
        wones[0:64, s * 32 + 2 * s] = -2.0
        wones[64:128, s * 32 + 2 * s + 1] = -2.0
        wones[0:64, (16 + s) * 32 + 2 * s] = 1.0
        wones[64:128, (16 + s) * 32 + 2 * s + 1] = 1.0
    cmask = np.zeros((1, 8 * 32), np.float32)
    for rr in range(2):
        for s in range(16):
            for g in range(4):
                p = rr * 64 + s * 4 + g
                if not _is_act_pair(p):
                    cb = rr * 4 + g
                    cmask[0, cb * 32 + 2 * s] = 1.0
                    cmask[0, cb * 32 + 2 * s + 1] = 1.0
    shared = {
        "wk1": Wk1.astype(np.float16), "bk1": bk1.reshape(D, 1),
        "wk2d": np.concatenate([Wk2, Wk2], axis=1).astype(np.float16),
        "bk2d": np.concatenate([bk2, bk2]).reshape(128, 1),
        "wq1": Wq1.astype(np.float16), "bq1": bq1.reshape(D, 1),
        "wq2": Wq2.astype(np.float16),
        "bq2d": np.concatenate([bq2, bq2]).reshape(128, 1),
        "wones": wones.astype(np.float16), "cmask": cmask.astype(np.float16),
        "ones64": np.ones((D, 1), np.float16),
        "ident": np.eye(128, dtype=ml_dtypes.bfloat16),
        "ident32": np.eye(128, dtype=np.float32),
    }
    shared = {k: np.ascontiguousarray(v) for k, v in shared.items()}

    in_maps = []
    for c in range(NCORES):
        b, h = c // 2, c % 2
        m = dict(shared)
        m["x1t"] = np.ascontiguousarray(x1[b].T.astype(np.float16))
        m["x2t"] = np.ascontiguousarray(x2[b, h * QSH:(h + 1) * QSH].T.astype(np.float16))
        m["rv"] = np.ascontiguousarray(r[b].astype(ml_dtypes.bfloat16))
        in_maps.append(m)

    nc = _get_nc()
    trace = bool(os.environ.get("BASS_TRACE"))
    if trace:
        _install_ntff_shim()
    res = None
    for attempt in range(3):
        try:
            res = bass_utils.run_bass_kernel_spmd(
                nc, in_maps, core_ids=list(range(NCORES)), trace=trace)
            break
        except Exception:
            # transient NRT_EXEC_UNIT_UNRECOVERABLE failures have been
            # observed on this fabric; retry (compile results are cached)
            if attempt == 2:
                raise
            import time
            time.sleep(5)
    LAST_RESULT = res

    # reassemble: yout[rr, f, t] with psum row t = 32g + 2s + i2,
    # local query = i2*128 + rr*64 + s*4 + g
    t = np.arange(128)
    g = t // 32
    m = t % 32
    s = m // 2
    i2 = m % 2
    out = np.empty((B, NQ, D), np.float32)
    for c in range(NCORES):
        b, h = c // 2, c % 2
        yc = res.results[c]["yout"]          # [2, D, 128]
        sc = res.results[c]["sout"]          # [2, 2, 128]
        for rr in range(2):
            qloc = i2 * 128 + rr * 64 + s * 4 + g
            ssum = sc[rr, 0] + sc[rr, 1]
            out[b, h * QSH + qloc, :] = (yc[rr] / ssum[None, :]).T
    return out


# revision 27
# speedup vs baseline: 1.2517x; 1.0007x over previous
"""Laplace attention kernel for Trainium2, 8 NeuronCores.

Math (per batch b):
  k = MLP_k(x1[b])  [NK, D];  q = MLP_q(x2[b])  [NQ, D]
  dist[i,j] = sum_d |k[j,d] - q[i,d]|
  out = softmax_j(-dist) @ r[b]

Distribution: core c = (b, h) = (c//2, c%2): batch b, query-half h (256 queries).

Per-core algorithm:
  - MLPs run transposed on the PE: kT2 [128=(i2,d), NK] holds kT stacked twice,
    q2T [128=(i2,d), 128] holds qT for query pair p = (p, 128+p).
  - For each query pair p, a [128, NK] tile M_p is produced:
      min-form pairs (DVE):  M_p = min(kT2, q_p)        (tensor_scalar)
      abs-form pairs (ACT):  M_p = |kT2 - q_p|          (activation Abs)
    The PE reduces over the 128 partitions with a constant ones-block lhsT
    (coefficient -2 for min-form, +1 for abs-form), writing 2 PSUM rows per
    pair.  The reduction is COLUMN-TILED 4 ways: pair p -> col group
    g = p%4, psum rows 32g + 2s + {0,1} (s = slot), so 4 matmul streams run
    concurrently in the PE array (tile_position=(0, 32g)) - ~4x the serial
    column rate.  A K=1 correction matmul per (group, window) adds
    A_j = sum_d k[j,d] to min-form rows.  The remaining B_i offset is
    row-constant and cancels in softmax.
  - softmax: ACT Exp (scale=-1) per 512-window with accum_out row-sums;
    weights bf16.  Division by the sum happens on the host.
  - value matmul: PE-transpose the bf16 weights to [j, q] layout (8x
    [128,128]), evacuate to SBUF, then 8 accumulating matmuls with r as
    stationary operand -> [D, 128] PSUM; written out, transposed on host.
"""

import os
import numpy as np
import ml_dtypes

import concourse.bass as bass
import concourse.mybir as mybir
from concourse.tile import TileContext
from concourse import bass_utils

B, NQ, NK, D = 4, 512, 1024, 64
NCORES = 8
QSH = NQ // 2           # queries per core
NPAIR = QSH // 2        # 128 query pairs per core
NWIN = NK // 512        # 512-column matmul windows
NG = 4                  # PE column groups (tile_position)
NS = 16                 # slots per group per round

F32 = mybir.dt.float32
F16 = mybir.dt.float16
BF16 = mybir.dt.bfloat16

LAST_RESULT = None      # BassKernelResults of the most recent run (for test.py)


def _is_act_pair(p):
    # pair p = rr*64 + s*4 + g ; put col-group 3 on ACT except the earliest
    # slot of round 0 (ACT is still evacuating the MLPs / kt2 not complete),
    # plus a few g==2 pairs to balance the measured DVE/ACT rates
    rr, m = divmod(p, 64)
    s, g = divmod(m, 4)
    if (rr, s, g) in ((0, 8, 2), (1, 4, 2), (1, 12, 2)):
        return True
    return g == 3 and not (rr == 0 and s < 1)


# ---------------------------------------------------------------------------
# walrus workaround: the CTRL-class instructions (Drain etc.) can carry only a
# few sem waits; hoist excess waits onto injected NoOps on the same engine.
def _split_excess_waits(nc, max_waits=1):
    for f in nc.m.functions:
        for bb in f.blocks:
            new_insts = []
            for inst in bb.instructions:
                si = inst.sync_info
                if si is not None and si.on_wait and len(si.on_wait) > max_waits:
                    waits = list(si.on_wait)
                    excess, keep = waits[:-max_waits], waits[-max_waits:]
                    for i in range(0, len(excess), max_waits):
                        nop = mybir.InstNoOp(
                            name=f"{inst.name}_waitsplit_{i // max_waits}",
                            ins=[], outs=[])
                        nop.engine = inst.engine
                        nop.sync_info = mybir.SyncInfo(
                            on_wait=excess[i:i + max_waits], on_update=[])
                        new_insts.append(nop)
                    si.on_wait = keep
                new_insts.append(inst)
            bb.instructions = new_insts


# shim antenv.axon_hooks (absent in this image) so BASS_TRACE=1 profiling works
def _install_ntff_shim():
    import sys, types
    if 'antenv.axon_hooks' in sys.modules:
        return
    try:
        mod = types.ModuleType('antenv.axon_hooks')
        state = {}
        mod.set_axon_ntff_profile_hook = lambda h: state.__setitem__('h', h)
        mod.get_axon_ntff_profile_hook = lambda: state.get('h')
        sys.modules['antenv.axon_hooks'] = mod
        import antenv
        antenv.axon_hooks = mod
        from trn_agent_boot.trn_boot import _ntff_profile_via_ctypes
        h = _ntff_profile_via_ctypes('/opt/axon/libaxon_pjrt.so')
        if h is not None:
            mod.set_axon_ntff_profile_hook(h)
    except Exception:
        pass


# ---------------------------------------------------------------------------
def _build_program():
    nc = bass.Bass("TRN2")

    x1t = nc.dram_tensor("x1t", [D, NK], F16, kind="ExternalInput")
    x2t = nc.dram_tensor("x2t", [D, QSH], F16, kind="ExternalInput")
    rv = nc.dram_tensor("rv", [NK, D], BF16, kind="ExternalInput")
    wk1 = nc.dram_tensor("wk1", [D, D], F16, kind="ExternalInput")
    wk2d = nc.dram_tensor("wk2d", [D, 128], F16, kind="ExternalInput")
    wq1 = nc.dram_tensor("wq1", [D, D], F16, kind="ExternalInput")
    wq2 = nc.dram_tensor("wq2", [D, D], F16, kind="ExternalInput")
    # bias4 cols: 0=bk1 (rows 0:64), 1=bk2 doubled, 2=bq1 (rows 0:64), 3=bq2 doubled
    bias4 = nc.dram_tensor("bias4", [128, 4], F32, kind="ExternalInput")
    wones = nc.dram_tensor("wones", [128, 32 * 32], F16, kind="ExternalInput")
    cmask = nc.dram_tensor("cmask", [1, 8 * 32], F16, kind="ExternalInput")
    ones64 = nc.dram_tensor("ones64", [D, 1], F16, kind="ExternalInput")
    ident = nc.dram_tensor("ident", [128, 128], BF16, kind="ExternalInput")
    ident32 = nc.dram_tensor("ident32", [128, 128], F32, kind="ExternalInput")
    yout = nc.dram_tensor("yout", [2, D, 128], F32, kind="ExternalOutput")
    sout = nc.dram_tensor("sout", [2, 2, 128], F32, kind="ExternalOutput")

    ACT = mybir.ActivationFunctionType
    ALU = mybir.AluOpType

    with TileContext(nc) as tc:
        import contextlib
        with contextlib.ExitStack() as ctx:
            consts = ctx.enter_context(tc.tile_pool(name="consts", bufs=1))

            x1t_sb = consts.tile([D, NK], F16)
            x2t_sb = consts.tile([D, QSH], F16)
            r_sb = consts.tile([128, 8 * D], BF16)
            wk1_sb = consts.tile([D, D], F16)
            wk2d_sb = consts.tile([D, 128], F16)
            wq1_sb = consts.tile([D, D], F16)
            wq2_sb = consts.tile([D, D], F16)
            bias4_sb = consts.tile([128, 4], F32)
            wones_sb = consts.tile([128, 32 * 32], F16)
            cmask_sb = consts.tile([1, 8 * 32], F16)
            ones64_sb = consts.tile([D, 1], F16)
            ident_sb = consts.tile([128, 128], BF16)
            ident32_sb = consts.tile([128, 128], F32)
            dummy_sb = consts.tile([1, 16], F32)

            nc.vector.memset(dummy_sb[:], 0.0)

            # input DMAs: 3 hw queues (sync / scalar=ACT / gpsimd); transfers
            # serialize per queue, so the gating operands (x2t, wq1, x1t w0)
            # each lead their own queue
            nc.sync.dma_start(out=x2t_sb[:], in_=x2t[:, :])
            nc.sync.dma_start(out=wq2_sb[:], in_=wq2[:, :])
            nc.sync.dma_start(out=wones_sb[:], in_=wones[:, :])
            nc.scalar.dma_start(out=wq1_sb[:], in_=wq1[:, :])
            nc.scalar.dma_start(out=bias4_sb[:], in_=bias4[:, :])
            # trigger the ACT exp-table load (~1.3us) right behind the two
            # critical scalar-queue DMAs, ahead of the first real activation
            nc.scalar.activation(dummy_sb[:], dummy_sb[:],
                                 mybir.ActivationFunctionType.Exp,
                                 bias=0.0, scale=1.0)
            nc.gpsimd.dma_start(out=x1t_sb[:, 0:512], in_=x1t[:, 0:512])
            nc.gpsimd.dma_start(out=wk1_sb[:], in_=wk1[:, :])
            nc.gpsimd.dma_start(out=wk2d_sb[:], in_=wk2d[:, :])
            nc.gpsimd.dma_start(out=x1t_sb[:, 512:1024], in_=x1t[:, 512:1024])
            nc.gpsimd.dma_start(out=cmask_sb[:], in_=cmask[:, :])
            nc.gpsimd.dma_start(out=ones64_sb[:], in_=ones64[:, :])
            nc.gpsimd.dma_start(out=ident_sb[:], in_=ident[:, :])
            nc.gpsimd.dma_start(out=ident32_sb[:], in_=ident32[:, :])
            for jt in range(8):
                nc.gpsimd.dma_start(out=r_sb[:, jt * D:(jt + 1) * D],
                                    in_=rv[jt * 128:(jt + 1) * 128, :])

            kt2_sb = consts.tile([128, NK], F16)
            q2t_sb = consts.tile([128, 128], F32)
            ht_sb = consts.tile([D, NK], F16)
            hqt_sb = consts.tile([D, QSH], F16)
            arow_sb = consts.tile([1, NK], F16)

            # ---- MLPs (transposed) ----
            # q-path and k-window-0 interleaved so PE and ACT ping-pong with
            # no serial DMA wait; all evacuations on ACT so the DVE can start
            # min production immediately after.
            with tc.tile_pool(name="mlppsum", bufs=2, space="PSUM") as mp:
                phq = mp.tile([D, QSH], F32, tag="ph")
                nc.tensor.matmul(phq[:], wq1_sb[:], x2t_sb[:], start=True, stop=True)
                ph0 = mp.tile([D, 512], F32, tag="ph")
                nc.tensor.matmul(ph0[:], wk1_sb[:], x1t_sb[:, 0:512],
                                 start=True, stop=True)
                nc.scalar.activation(hqt_sb[:], phq[:], ACT.Relu,
                                     bias=bias4_sb[0:64, 2:3], scale=1.0)
                # k-path L1 relu on the (pre-generation) idle DVE:
                # (psum + bk1) max 0
                nc.vector.tensor_scalar(ht_sb[:, 0:512], ph0[:],
                                        bias4_sb[0:64, 0:1], 0.0,
                                        ALU.add, ALU.max)
                pq = mp.tile([128, 128], F32, tag="pk")
                nc.tensor.matmul(pq[0:64, :], wq2_sb[:], hqt_sb[:, 0:128],
                                 start=True, stop=False, skip_group_check=True)
                nc.tensor.matmul(pq[64:128, :], wq2_sb[:], hqt_sb[:, 128:256],
                                 start=True, stop=True, skip_group_check=True)
                pk0 = mp.tile([128, 512], F32, tag="pk")
                nc.tensor.matmul(pk0[:], wk2d_sb[:], ht_sb[:, 0:512],
                                 start=True, stop=True)
                nc.vector.tensor_scalar(q2t_sb[:], pq[:],
                                        bias4_sb[:, 3:4], None, ALU.add)
                nc.scalar.activation(kt2_sb[:, 0:512], pk0[:],
                                     ACT.Identity, bias=bias4_sb[:, 1:2], scale=1.0)
                ph1 = mp.tile([D, 512], F32, tag="ph")
                nc.tensor.matmul(ph1[:], wk1_sb[:], x1t_sb[:, 512:1024],
                                 start=True, stop=True)
                nc.vector.tensor_scalar(ht_sb[:, 512:1024], ph1[:],
                                        bias4_sb[0:64, 0:1], 0.0,
                                        ALU.add, ALU.max)
                pk1 = mp.tile([128, 512], F32, tag="pk")
                nc.tensor.matmul(pk1[:], wk2d_sb[:], ht_sb[:, 512:1024],
                                 start=True, stop=True)
                nc.scalar.activation(kt2_sb[:, 512:1024], pk1[:],
                                     ACT.Identity, bias=bias4_sb[:, 1:2], scale=1.0)
                # A_j = sum_d k[j, d] (same fp16 k the min path sees)
                pa = mp.tile([1, NK], F32, tag="pa")
                for w in range(NWIN):
                    nc.tensor.matmul(pa[:, w * 512:(w + 1) * 512], ones64_sb[:],
                                     kt2_sb[0:64, w * 512:(w + 1) * 512],
                                     start=True, stop=True, skip_group_check=True)
                nc.scalar.copy(arow_sb[:], pa[:])

            # ---- main loop ----
            mpool = ctx.enter_context(tc.tile_pool(name="mtiles", bufs=8))
            dpool = ctx.enter_context(
                tc.tile_pool(name="dist", bufs=2, space="PSUM"))
            opool = ctx.enter_context(
                tc.tile_pool(name="outp", bufs=2, space="PSUM"))
            vpool = ctx.enter_context(
                tc.tile_pool(name="vps", bufs=1, space="PSUM"))
            spool = ctx.enter_context(tc.tile_pool(name="smax", bufs=2))
            otpool = ctx.enter_context(tc.tile_pool(name="outs", bufs=2))

            def make_tail(rr, dists):
                state = {}

                def exp_half(w):
                    if w == 0:
                        state["expw"] = spool.tile([128, NK], BF16, name="expw", tag="expw")
                        state["ssum"] = spool.tile([128, 2], F32, name="ssum", tag="ssum")
                        state["expt"] = spool.tile([128, 8 * 128], BF16, name="expt", tag="expt")
                    nc.scalar.activation(
                        state["expw"][:, w * 512:(w + 1) * 512],
                        dists[w][:], ACT.Exp,
                        bias=0.0, scale=-1.0,
                        accum_out=state["ssum"][:, w:w + 1])

                def tp_pair(h):
                    # transpose 2 of the 8 [128,128] blocks on the PE;
                    # evacuations split DVE/ACT so the tp psum ring (bufs=2)
                    # turns over fast enough not to stall the PE queue
                    expw, expt = state["expw"], state["expt"]
                    for jt in (2 * h, 2 * h + 1):
                        tp = opool.tile([128, 128], BF16, tag="tp")
                        nc.tensor.transpose(tp[:], expw[:, jt * 128:(jt + 1) * 128],
                                            ident_sb[:])
                        if jt % 2 == 0:
                            nc.vector.tensor_copy(
                                expt[:, jt * 128:(jt + 1) * 128], tp[:])
                        else:
                            nc.scalar.copy(
                                expt[:, jt * 128:(jt + 1) * 128], tp[:])

                def value():
                    expt = state["expt"]
                    ssum = state["ssum"]
                    # partition-dim ssum -> free-dim via PE transpose, then a
                    # linear 2-partition DMA (a direct [128,1]-strided DMA
                    # costs 128 descriptors and lands ~5us late)
                    stp = vpool.tile([2, 128], F32, tag="stp")
                    nc.tensor.transpose(stp[:], ssum[:, 0:2], ident32_sb[:])
                    sst = otpool.tile([2, 128], F32, tag="sst")
                    nc.scalar.copy(sst[:], stp[:])
                    nc.gpsimd.dma_start(out=sout[rr, :, :], in_=sst[:])
                    out_ps = vpool.tile([D, 128], F32, tag="ops")
                    for jt in range(8):
                        nc.tensor.matmul(out_ps[:, :],
                                         r_sb[:, jt * D:(jt + 1) * D],
                                         expt[:, jt * 128:(jt + 1) * 128],
                                         start=(jt == 0), stop=(jt == 7),
                                         skip_group_check=True)
                    ot = otpool.tile([D, 128], F32, tag="ot")
                    nc.scalar.copy(ot[:], out_ps[:])
                    nc.sync.dma_start(out=yout[rr, :, :], in_=ot[:])
                return exp_half, tp_pair, value

            prev = None
            for rr in range(2):
                dists = [dpool.tile([128, 512], F32, name=f"dist{w}",
                                    tag=f"dist{w}") for w in range(NWIN)]
                for s in range(NS):
                    for g in range(NG):
                        p = rr * 64 + s * 4 + g
                        mt = mpool.tile([128, NK], F16, tag="mt")
                        if _is_act_pair(p):
                            nc.scalar.activation(mt[:], kt2_sb[:], ACT.Abs,
                                                 bias=q2t_sb[:, p:p + 1], scale=-1.0)
                            bi = 16 + s
                        elif rr == 0 and s < 2:
                            # per-window halves: lets window-0 matmuls start
                            # before the second kt2 window is computed
                            for w in range(NWIN):
                                nc.vector.tensor_scalar(
                                    mt[:, w * 512:(w + 1) * 512],
                                    kt2_sb[:, w * 512:(w + 1) * 512],
                                    q2t_sb[:, p:p + 1], None, ALU.min)
                            bi = s
                        else:
                            nc.vector.tensor_scalar(mt[:], kt2_sb[:],
                                                    q2t_sb[:, p:p + 1], None, ALU.min)
                            bi = s
                        for w in range(NWIN):
                            nc.tensor.matmul(
                                dists[w][32 * g:32 * g + 32, :],
                                wones_sb[:, bi * 32:(bi + 1) * 32],
                                mt[:, w * 512:(w + 1) * 512],
                                start=(s == 0), stop=False,
                                skip_group_check=True,
                                tile_position=(0, 32 * g))
                    if prev is not None:
                        if s in (1, 2, 3, 4):
                            prev[1](s - 1)    # prev-round transposes
                        elif s == 6:
                            prev[2]()         # value matmuls + out DMA
                            prev = None
                # A_j correction (min-form rows only), closes each psum
                # window; exp of a window issues as soon as it closes
                cur = make_tail(rr, dists)
                for w in range(NWIN):
                    for g in range(NG):
                        cb = rr * 4 + g
                        nc.tensor.matmul(
                            dists[w][32 * g:32 * g + 32, :],
                            cmask_sb[:, cb * 32:(cb + 1) * 32],
                            arow_sb[:, w * 512:(w + 1) * 512],
                            start=False, stop=True,
                            skip_group_check=True,
                            tile_position=(0, 32 * g))
                    cur[0](w)
                prev = cur
            # tail of the final round (exp halves already issued above)
            for h in range(4):
                prev[1](h)
            prev[2]()

    _split_excess_waits(nc)
    return nc


_NC_CACHE = None


def _get_nc():
    global _NC_CACHE
    if _NC_CACHE is None:
        _NC_CACHE = _build_program()
    return _NC_CACHE


def kernel(x1, x2, r, Wk1, bk1, Wk2, bk2, Wq1, bq1, Wq2, bq2):
    global LAST_RESULT
    x1 = np.asarray(x1, np.float32)
    x2 = np.asarray(x2, np.float32)
    r = np.asarray(r, np.float32)
    Wk1 = np.asarray(Wk1, np.float32); bk1 = np.asarray(bk1, np.float32)
    Wk2 = np.asarray(Wk2, np.float32); bk2 = np.asarray(bk2, np.float32)
    Wq1 = np.asarray(Wq1, np.float32); bq1 = np.asarray(bq1, np.float32)
    Wq2 = np.asarray(Wq2, np.float32); bq2 = np.asarray(bq2, np.float32)

    # constant PE weights: ones-block lhsT [128, 32] per (form, slot); block
    # b = form*16 + s maps partition halves to psum rows (2s, 2s+1) within a
    # col group, coefficient -2 (min-form) / +1 (abs-form). cmask: per
    # (round, group) A_j-correction masks over the group's 32 psum rows.
    wones = np.zeros((128, 32 * 32), np.float32)
    for s in range(16):
        wones[0:64, s * 32 + 2 * s] = -2.0
        wones[64:128, s * 32 + 2 * s + 1] = -2.0
        wones[0:64, (16 + s) * 32 + 2 * s] = 1.0
        wones[64:128, (16 + s) * 32 + 2 * s + 1] = 1.0
    cmask = np.zeros((1, 8 * 32), np.float32)
    for rr in range(2):
        for s in range(16):
            for g in range(4):
                p = rr * 64 + s * 4 + g
                if not _is_act_pair(p):
                    cb = rr * 4 + g
                    cmask[0, cb * 32 + 2 * s] = 1.0
                    cmask[0, cb * 32 + 2 * s + 1] = 1.0
    bias4 = np.zeros((128, 4), np.float32)
    bias4[0:64, 0] = bk1
    bias4[:, 1] = np.concatenate([bk2, bk2])
    bias4[0:64, 2] = bq1
    bias4[:, 3] = np.concatenate([bq2, bq2])
    shared = {
        "wk1": Wk1.astype(np.float16),
        "wk2d": np.concatenate([Wk2, Wk2], axis=1).astype(np.float16),
        "wq1": Wq1.astype(np.float16),
        "wq2": Wq2.astype(np.float16),
        "bias4": bias4,
        "wones": wones.astype(np.float16), "cmask": cmask.astype(np.float16),
        "ones64": np.ones((D, 1), np.float16),
        "ident": np.eye(128, dtype=ml_dtypes.bfloat16),
        "ident32": np.eye(128, dtype=np.float32),
    }
    shared = {k: np.ascontiguousarray(v) for k, v in shared.items()}

    in_maps = []
    for c in range(NCORES):
        b, h = c // 2, c % 2
        m = dict(shared)
        m["x1t"] = np.ascontiguousarray(x1[b].T.astype(np.float16))
        m["x2t"] = np.ascontiguousarray(x2[b, h * QSH:(h + 1) * QSH].T.astype(np.float16))
        m["rv"] = np.ascontiguousarray(r[b].astype(ml_dtypes.bfloat16))
        in_maps.append(m)

    nc = _get_nc()
    trace = bool(os.environ.get("BASS_TRACE"))
    if trace:
        _install_ntff_shim()
    res = None
    for attempt in range(3):
        try:
            res = bass_utils.run_bass_kernel_spmd(
                nc, in_maps, core_ids=list(range(NCORES)), trace=trace)
            break
        except Exception:
            # transient NRT_EXEC_UNIT_UNRECOVERABLE failures have been
            # observed on this fabric; retry (compile results are cached)
            if attempt == 2:
                raise
            import time
            time.sleep(5)
    LAST_RESULT = res

    # reassemble: yout[rr, f, t] with psum row t = 32g + 2s + i2,
    # local query = i2*128 + rr*64 + s*4 + g
    t = np.arange(128)
    g = t // 32
    m = t % 32
    s = m // 2
    i2 = m % 2
    out = np.empty((B, NQ, D), np.float32)
    for c in range(NCORES):
        b, h = c // 2, c % 2
        yc = res.results[c]["yout"]          # [2, D, 128]
        sc = res.results[c]["sout"]          # [2, 2, 128]
        for rr in range(2):
            qloc = i2 * 128 + rr * 64 + s * 4 + g
            ssum = sc[rr, 0] + sc[rr, 1]
            out[b, h * QSH + qloc, :] = (yc[rr] / ssum[None, :]).T
    return out
